# revision 5
# baseline (speedup 1.0000x reference)
"""Trainium2 Bass kernel for nn_AttentionLayer (attention pooling).

Reference math (per batch row b):
    u   = tanh(x[b] @ W + b_vec)        # [T, M]
    s   = u @ us                        # [T]
    a   = softmax(s) * mask / sum       # [T]  (mask is all ones per spec)
    out = a @ x[b]                      # [D]

Strategy: data-parallel over batch, B=32 rows -> 4 rows per NeuronCore on
8 cores.  x is converted to bf16 on the HOST (the device math is bf16
anyway), halving HBM traffic.  The x-transpose needed by the GEMM is done
entirely on the Vector engine (32x32 stream-transpose blocks), freeing
the PE from 512 transpose matmuls.  Per core, per row:
  - x tiles [128t, 1024d] are DMA'd bf16; DVE stream-transposes each into
    z[32a+r, t, 32j+s] = x[128t+32a+s, 32j+r] (in-place 32x32 blocks);
  - the GEMM runs ROW-TILED: W rows are replicated into Wrep[32a+r, j, m]
    = W[32j+r, m]; per d-slice j ONE full-array LDWEIGHTS loads all four
    row-groups' weights, then 4 non-self-loading matmuls (tile_position
    (32a,0), K=32, N=512) contract slice j for all tokens concurrently.
    Each row-group accumulates into its OWN psum bank (concurrent
    row-tiled matmuls into one bank wedge the device);
  - tanh+bias on ScalarE scatters bank a to u^T[m, t=q*512+tl*128+32a+s];
  - scores per t-chunk via matmul(lhsT=u chunk, rhs=us) -> [128t, 1];
  - exp on ScalarE with accum_out row sums; Sum(e) is reduced by a
    ones-matmul, inverted on DVE, broadcast back through a 1xK matmul,
    and e is pre-scaled by 1/Sum(e) so pooling emits normalized output;
  - pooling = four COLUMN-TILED matmuls per t-chunk (tile_position
    (0,32g), N=256 each) streaming concurrently on separate XBUSes.
Pooling of row r is emitted during row r+1 so the PE queue never blocks
on the exp/normalize chain.
"""
import numpy as np
import ml_dtypes

import concourse.bacc as bacc
import concourse.mybir as mybir
from concourse.tile import TileContext
from concourse.bass_utils import run_bass_kernel_spmd

F32 = mybir.dt.float32
BF16 = mybir.dt.bfloat16

B, T, D, M = 32, 2048, 1024, 128
NCORES = 8
B_SH = B // NCORES   # 4 batch rows per core
P = 128
NT = T // P          # 16 t-tiles per row
NJ = 32              # d-slices of 32
DG = D // 4          # 256 columns per pooling col-group


def _matmul_noldw(nc, out, lhsT, rhs, start, stop, tile_position):
    """Row-tiled matmul that reuses the stationary weights loaded by a
    preceding nc.tensor.ldweights (full-array load) instead of reloading."""
    te = nc.tensor
    ifmap_ap = te.lower_ap(rhs.opt({0}), opt=False)
    weights_ap = te.lower_ap(lhsT.opt({0}), opt=False, for_matmul_weights=True)
    out_ap = te.lower_ap(out)
    inst = mybir.InstMatmult(
        name=nc.get_next_instruction_name(),
        replication_resolution=0,
        replication_shift_amnt=0,
        replication_num_rows=0,
        start_tensor_calc=start,
        stop_tensor_calc=stop,
        ins=[ifmap_ap, weights_ap],
        outs=[out_ap],
        perf_mode=None,
        is_transpose=None,
        ifmap_quant_offset=None,
        weights_quant_offset=None,
        bass_skip_group_check=False,
        tile_position=tile_position,
        tile_size=(32, 128),
        ldweights=False,
    )
    return te.add_instruction(inst)


def _build_nc():
    nc = bacc.Bacc("TRN2", target_bir_lowering=False, debug=False,
                   num_devices=NCORES)
    x = nc.declare_dram_parameter("x", [B_SH, T, D], BF16, isOutput=False)
    W = nc.declare_dram_parameter("W", [D, M], BF16, isOutput=False)
    b = nc.declare_dram_parameter("b", [M], F32, isOutput=False)
    us = nc.declare_dram_parameter("us", [M, 1], BF16, isOutput=False)
    y = nc.declare_dram_parameter("y", [B_SH, D], F32, isOutput=True)

    with TileContext(nc) as tc:
        with (
            tc.tile_pool(name="singles", bufs=1) as singles,
            tc.tile_pool(name="xb", bufs=3) as xb_pool,
            tc.tile_pool(name="z", bufs=2) as z_pool,
            tc.tile_pool(name="u", bufs=2) as u_pool,
            tc.tile_pool(name="e", bufs=2) as e_pool,
            tc.tile_pool(name="u_ps", bufs=4, space="PSUM") as u_psum,
            tc.tile_pool(name="s_ps", bufs=1, space="PSUM") as s_psum,
            tc.tile_pool(name="o_ps", bufs=2, space="PSUM") as o_psum,
        ):
            # constants (DVE memsets only; no gpsimd library wait)
            wones = singles.tile([P, P], BF16)
            nc.vector.memset(wones, 1.0)
            ones_col = singles.tile([P, 1], F32)
            nc.vector.memset(ones_col, 1.0)
            ones_row = singles.tile([1, P], F32)
            nc.vector.memset(ones_row, 1.0)

            x_tiles = {}

            def start_row_dmas(r):
                xb = xb_pool.tile([P, NT, D], BF16, tag="xb", name=f"xb_{r}")
                src = x[r].rearrange("(n p) d -> p n d", p=P)
                for t in range(NT):
                    nc.sync.dma_start(out=xb[:, t, :], in_=src[:, t, :])
                x_tiles[r] = xb

            start_row_dmas(0)

            # Wrep[32a+r, j, m] = W[32j+r, m]
            w_sb = singles.tile([P, NJ, M], BF16)
            wsrc = W.rearrange("(j r) m -> r j m", r=32)
            for a in range(4):
                nc.sync.dma_start(out=w_sb[32 * a:32 * a + 32, :, :], in_=wsrc)
            b_sb = singles.tile([P, 1], F32)
            nc.sync.dma_start(out=b_sb, in_=b.rearrange("(p o) -> p o", o=1))
            us_bf = singles.tile([P, 1], BF16)
            nc.sync.dma_start(out=us_bf, in_=us[:, :])

            # PE warm-up while the first DMAs stream (HAM un-throttle)
            warm = u_psum.tile([P, 4, 4, 32], F32, tag="ub", name="warm")
            wv = warm.rearrange("p a b c -> p (a b c)")
            for _ in range(24):
                nc.tensor.matmul(wv[:, 0:P], wones, wones,
                                 start=True, stop=True)

            pending_pool = []

            def emit_pooling(r, e_scaled, o_tile):
                for j in range(NT):
                    for g in range(4):
                        nc.tensor.matmul(
                            o_tile[32 * g:32 * g + 1, :DG],
                            e_scaled[:, j:j + 1],
                            x_tiles[r][:, j, g * DG:(g + 1) * DG],
                            start=(j == 0), stop=(j == NT - 1),
                            tile_position=(0, 32 * g),
                        )
                o_sb = e_pool.tile([P, DG], F32, tag="osb", name=f"osb_{r}")
                for g in range(4):
                    nc.scalar.copy(
                        out=o_sb[32 * g:32 * g + 1, :],
                        in_=o_tile[32 * g:32 * g + 1, :DG],
                    )
                    nc.sync.dma_start(
                        out=y[r:r + 1, g * DG:(g + 1) * DG],
                        in_=o_sb[32 * g:32 * g + 1, :],
                    )

            for r in range(B_SH):
                xb = x_tiles[r]
                if r + 1 < B_SH:
                    start_row_dmas(r + 1)

                # DVE stream-transpose each t-tile:
                # z[32a+r', t, 32j+s] = x[t*128+32a+s, 32j+r']
                z_sb = z_pool.tile([P, NT, D], BF16, tag="z", name=f"z_{r}")
                for t in range(NT):
                    nc.vector.transpose(out=z_sb[:, t, :], in_=xb[:, t, :])
                zv = z_sb.rearrange("p n (j s) -> p n j s", s=32)

                u_sb = u_pool.tile([P, T], BF16, tag="u", name=f"u_{r}")
                sp = s_psum.tile([P, NT + 2], F32, tag="s")
                rs = e_pool.tile([P, 1], F32, tag="rs", name=f"rs_{r}")

                # row-tiled GEMM: one LDW per d-slice, 4 concurrent
                # row-group matmuls into 4 separate psum banks
                ubs = [u_psum.tile([P, 4, 4, 32], F32, tag="ub",
                                   name=f"ub{a}") for a in range(4)]
                for j in range(NJ):
                    nc.tensor.ldweights(w_sb[:, j, :])
                    for a in range(4):
                        _matmul_noldw(
                            nc, ubs[a][:, :, :, :],
                            w_sb[32 * a:32 * a + 32, j, :],
                            zv[32 * a:32 * a + 32, :, j, :],
                            start=(j == 0), stop=(j == NJ - 1),
                            tile_position=(32 * a, 0),
                        )

                # tanh + bias; scatter bank a to u^T tokens q*512+tl*128+32a+s
                uview = u_sb.rearrange("p (q tl g s) -> p q tl g s",
                                       q=4, tl=4, g=4)
                for a in range(4):
                    nc.scalar.activation(
                        out=uview[:, :, :, a, :],
                        in_=ubs[a],
                        func=mybir.ActivationFunctionType.Tanh,
                        bias=b_sb, scale=1.0,
                    )

                # pooling of the previous row fills the PE while tanh runs
                if pending_pool:
                    emit_pooling(*pending_pool.pop(0))

                for t in range(NT):
                    nc.tensor.matmul(
                        sp[:, t:t + 1],
                        u_sb[:, t * P:(t + 1) * P],
                        us_bf, start=True, stop=True,
                    )

                # softmax weights, pre-scaled by 1/sum
                e_pack = e_pool.tile([P, NT], BF16, tag="ep", name=f"ep_{r}")
                nc.scalar.activation(
                    out=e_pack, in_=sp[:, :NT],
                    func=mybir.ActivationFunctionType.Exp,
                    accum_out=rs,
                )
                nc.tensor.matmul(sp[0:1, NT:NT + 1], rs, ones_col,
                                 start=True, stop=True)
                tinv = e_pool.tile([1, 1], F32, tag="tinv", name=f"tinv_{r}")
                nc.vector.reciprocal(out=tinv, in_=sp[0:1, NT:NT + 1])
                nc.tensor.matmul(sp[:, NT + 1:NT + 2], ones_row, tinv,
                                 start=True, stop=True)
                e_scaled = e_pool.tile([P, NT], BF16, tag="es", name=f"es_{r}")
                nc.vector.tensor_scalar_mul(e_scaled, e_pack,
                                            sp[:, NT + 1:NT + 2])

                o_tile = o_psum.tile([P, DG], F32, tag="o")
                pending_pool.append((r, e_scaled, o_tile))

            while pending_pool:
                emit_pooling(*pending_pool.pop(0))

    nc.compile()
    return nc


_NC_CACHE = []


def _numpy_reference(x, W, b, us, mask):
    m = mask.astype(x.dtype)
    u = np.tanh(np.einsum('btd,dm->btm', x, W) + b)
    utu = np.einsum('btm,mo->bto', u, us)[..., 0]
    e = np.exp(utu - utu.max(axis=-1, keepdims=True))
    e = m * e
    a = e / e.sum(axis=-1, keepdims=True)
    return np.einsum('bt,btd->bd', a, x).astype(np.float32)


def _make_in_maps(x, W, b, us):
    x_bf = np.ascontiguousarray(x).astype(ml_dtypes.bfloat16)
    W_bf = np.ascontiguousarray(W).astype(ml_dtypes.bfloat16)
    us_bf = np.ascontiguousarray(us).astype(ml_dtypes.bfloat16)
    b_f = np.ascontiguousarray(b).astype(np.float32)
    in_maps = []
    for i in range(NCORES):
        in_maps.append({
            "x": np.ascontiguousarray(x_bf[i * B_SH:(i + 1) * B_SH]),
            "W": W_bf, "b": b_f, "us": us_bf,
        })
    return in_maps


def kernel(x, W, b, us, mask):
    x = np.asarray(x, dtype=np.float32)
    W = np.asarray(W, dtype=np.float32)
    b = np.asarray(b, dtype=np.float32)
    us = np.asarray(us, dtype=np.float32)
    mask = np.asarray(mask)

    if not bool(mask.all()):
        # spec guarantees an all-ones mask; fall back to exact numpy
        # reference if that ever changes
        return _numpy_reference(x, W, b, us, mask)

    if not _NC_CACHE:
        _NC_CACHE.append(_build_nc())
    nc = _NC_CACHE[0]

    res = run_bass_kernel_spmd(nc, _make_in_maps(x, W, b, us),
                               core_ids=list(range(NCORES)), trace=False)
    return np.concatenate([res.results[i]["y"] for i in range(NCORES)], axis=0)
